# revision 1
# baseline (speedup 1.0000x reference)
"""Trainium2 Bass kernel for nn_CrossAtt (dual cross-attention + 3x3 conv + BN + ReLU).

Sharding: 8 cores = (sample s in 0..3) x (h-half in 0..1). Each core computes
its 32 output rows plus a 1-row attention halo on each side (34 rows = 2176
query positions, host-zero-padded so the program is SPMD-uniform), then runs
the 3x3 conv locally. No collectives.

Device layout choices:
- scoresT [m, n] comes straight off the PE (lhsT=k zero-padded to K=128,
  rhs=q), so softmax needs no transpose of the 4096x2176 matrix.
- exp on ScalarE (no max subtraction; |scores| <~ 5 so fp32 exp is safe).
- AV: out^T[n, 257] = expT.T @ [vT | ones]; col 256 accumulates the softmax
  denominator S for free.
- normalize by (gamma * mask / S) as a per-partition scalar; mask zeroes the
  fake padded query rows. PE-transposes the small [n,256] result to [256,n].
- residual + gamma*bv bias are folded into the host-prepared x?r inputs.
- conv3x3 = 9 shifted matmuls over a [512, 35*66] zero-padded cat buffer;
  BN+ReLU fused into one activation (scale=inv, bias=beta per partition).
"""
import sys

if "/opt/trn_rl_repo" not in sys.path:
    sys.path.insert(0, "/opt/trn_rl_repo")

import numpy as np

import concourse.bass as bass
import concourse.bacc as bacc
import concourse.mybir as mybir
import concourse.tile as tile
from concourse.bass import ds, ts
from concourse.bass_utils import run_bass_kernel_spmd

F32 = mybir.dt.float32
F32R = mybir.dt.float32r  # same bits as fp32; 1 cycle/row PE mode (vs 4 for fp32)
EPS = 1e-5
P = 128
C = 256          # channels
M = 4096         # key/value positions (64*64)
NQ = 2176        # query positions per core (34 rows * 64), host padded
NROWS = 35       # cat_pad rows (34 data + 1 zero)
WPAD = 66        # cat_pad row width (64 + 2 zero cols)
# all moving dims >= 256 so the fp32r fast path applies
ATT_BLOCKS = [(i * 256, 256) for i in range(8)] + [(2048, 128)]
QWINS = [(0, 512), (512, 512), (1024, 512), (1536, 384), (1920, 256)]
CONV_WINS = [(1, 512), (513, 512), (1025, 512), (1537, 318), (1855, 256)]

_CACHE = {}


def _wins(total, w):
    return [(i, min(w, total - i)) for i in range(0, total, w)]


def _mm(nc, out, lhsT, rhs, **kw):
    nc.tensor.matmul(out, lhsT, rhs, **kw)


def _declare_io(nc):
    t = {}
    inp = lambda name, shape, dt=F32: t.__setitem__(
        name, nc.dram_tensor(name, shape, dt, kind="ExternalInput"))
    out = lambda name, shape, dt=F32: t.__setitem__(
        name, nc.dram_tensor(name, shape, dt, kind="ExternalOutput"))
    # fp32r = same 32-bit data; matmul operands must be declared fp32r end-to-end
    inp("x1", [C, M], F32R); inp("x2", [C, M], F32R)
    inp("x1q", [C, NQ], F32R); inp("x2q", [C, NQ], F32R)
    inp("x1r", [C, NQ]); inp("x2r", [C, NQ])
    inp("maskg", [P, 17])
    inp("wq1T", [P, 2, 32], F32R); inp("wq2T", [P, 2, 32], F32R)
    inp("wk1T", [P, 2, 64], F32R); inp("wk2T", [P, 2, 64], F32R)
    inp("wv1T", [P, 2, C], F32R); inp("wv2T", [P, 2, C], F32R)
    inp("bq1", [32, 1]); inp("bq2", [32, 1])
    inp("bk1", [64, 1]); inp("bk2", [64, 1])
    inp("cinv", [P, 2]); inp("cbeta", [P, 2])
    inp("wct", [3, 3, 2 * C, C], F32R)
    inp("ident", [P, P])
    out("feat", [C, 32, 64]); out("o1", [C, 32, 64], F32R); out("o2", [C, 32, 64], F32R)
    return t


def _emit(nc, tc, t, ctx):
    big = ctx.enter_context(tc.tile_pool(name="big", bufs=3))
    kqp = ctx.enter_context(tc.tile_pool(name="kq", bufs=1))
    sing = ctx.enter_context(tc.tile_pool(name="sing", bufs=1))
    expp = ctx.enter_context(tc.tile_pool(name="expp", bufs=3))
    normp = ctx.enter_context(tc.tile_pool(name="normp", bufs=3))
    scalp = ctx.enter_context(tc.tile_pool(name="scalp", bufs=4))
    resp = ctx.enter_context(tc.tile_pool(name="resp", bufs=4))
    wcp = ctx.enter_context(tc.tile_pool(name="wcp", bufs=12))
    psA = ctx.enter_context(tc.tile_pool(name="psA", bufs=2, space="PSUM"))
    psS = ctx.enter_context(tc.tile_pool(name="psS", bufs=2, space="PSUM"))

    BIG_SHAPE_BYTES = [P, 4 * NROWS * WPAD]  # cat_pad is the largest big tile

    # ---- constants / weights to SBUF ----
    idt = sing.tile([P, P], F32)
    nc.sync.dma_start(out=idt, in_=t["ident"][:])
    wq_sb, wk_sb, wv_sb, bq_sb, bk_sb = {}, {}, {}, {}, {}
    for b in (1, 2):
        wq_sb[b] = sing.tile([P, 2, 32], F32R, tag=f"wq{b}", name=f"wq{b}")
        nc.sync.dma_start(out=wq_sb[b], in_=t[f"wq{b}T"][:])
        wk_sb[b] = sing.tile([P, 2, 64], F32R, tag=f"wk{b}", name=f"wk{b}")
        nc.sync.dma_start(out=wk_sb[b], in_=t[f"wk{b}T"][:])
        wv_sb[b] = sing.tile([P, 2, C], F32R, tag=f"wv{b}", name=f"wv{b}")
        nc.sync.dma_start(out=wv_sb[b], in_=t[f"wv{b}T"][:])
        bq_sb[b] = sing.tile([32, 1], F32, tag=f"bq{b}", name=f"bq{b}")
        nc.sync.dma_start(out=bq_sb[b], in_=t[f"bq{b}"][:])
        bk_sb[b] = sing.tile([64, 1], F32, tag=f"bk{b}", name=f"bk{b}")
        nc.sync.dma_start(out=bk_sb[b], in_=t[f"bk{b}"][:])
    cinv_sb = sing.tile([P, 2], F32, tag="cinv")
    nc.sync.dma_start(out=cinv_sb, in_=t["cinv"][:])
    cbeta_sb = sing.tile([P, 2], F32, tag="cbeta")
    nc.sync.dma_start(out=cbeta_sb, in_=t["cbeta"][:])
    maskg_sb = sing.tile([P, 17], F32, tag="maskg")
    nc.sync.dma_start(out=maskg_sb, in_=t["maskg"][:])

    # ---- load x1, x2 (two column-half DMAs so the PE can start earlier) ----
    def load_x(name):
        x_sb = big.tile(BIG_SHAPE_BYTES, F32R, tag="big")
        xv = x_sb[:, : 2 * M].rearrange("p (kc n) -> p kc n", kc=2)
        src_ap = t[name][:].rearrange("(kc p) n -> p kc n", p=P)
        for c0 in range(0, M, 1024):
            nc.sync.dma_start(out=xv[:, :, ds(c0, 1024)],
                              in_=src_ap[:, :, ds(c0, 1024)])
        return xv

    x1_sb = load_x("x1")
    x2_sb = load_x("x2")

    # ---- k projections: k_b = wk_b @ x_b + bk_b, stored [128(c pad0), 4096] ----
    k_sb = {}
    for b, x_sb in ((1, x1_sb), (2, x2_sb)):
        kp = kqp.tile([P, M], F32R, tag=f"k{b}")
        for w0, ww in _wins(M, 512):
            ps = psS.tile([P, 1024], F32, tag="sc")
            for kc in range(2):
                _mm(nc, ps[0:64, :ww], wk_sb[b][:, kc, :],
                    x_sb[:, kc, ds(w0, ww)],
                    start=(kc == 0), stop=(kc == 1))
            nc.vector.tensor_scalar_add(kp[0:64, ds(w0, ww)], ps[0:64, :ww], bk_sb[b])
        k_sb[b] = kp

    # ---- vT projections: vT_b[m, c] = x_b.T @ wv_bT (no bias), plus ones col ----
    def make_vt(x_sb, b):
        vt = big.tile(BIG_SHAPE_BYTES, F32R, tag="big")
        vtv = vt[:, : 32 * 258].rearrange("p (mi c) -> p mi c", mi=32)
        nc.vector.memset(vtv[:, :, 256:257].bitcast(F32), 1.0)
        nc.vector.memset(vtv[:, :, 257:258].bitcast(F32), 0.0)
        for mi in range(32):
            ps_full = psS.tile([P, 1024], F32, tag="sc", name="vtps")
            ps = ps_full[:, :256]
            for kc in range(2):
                _mm(nc, ps, x_sb[:, kc, ts(mi, P)], wv_sb[b][:, kc, :],
                    start=(kc == 0), stop=(kc == 1))
            nc.vector.tensor_copy(out=vtv[:, mi, 0:256], in_=ps)
        return vtv

    # ---- q projection (shared by both branches): qp [128(c pad0), 2176] ----
    qp = kqp.tile([P, NQ], F32R, tag="qp")

    def q_half(name, b, row0):
        xq = big.tile(BIG_SHAPE_BYTES, F32R, tag="big")
        xqv = xq[:, : 2 * NQ].rearrange("p (kc n) -> p kc n", kc=2)
        xq_src = t[name][:].rearrange("(kc p) n -> p kc n", p=P)
        nc.sync.dma_start(out=xqv[:, :, 0:1088], in_=xq_src[:, :, 0:1088])
        nc.sync.dma_start(out=xqv[:, :, 1088:NQ], in_=xq_src[:, :, 1088:NQ])
        for w0, ww in QWINS:
            ps = psS.tile([P, 1024], F32, tag="sc")
            for kc in range(2):
                _mm(nc, ps[0:32, :ww], wq_sb[b][:, kc, :],
                    xqv[:, kc, ds(w0, ww)],
                    start=(kc == 0), stop=(kc == 1))
            nc.vector.tensor_scalar_add(qp[row0:row0 + 32, ds(w0, ww)],
                                        ps[0:32, :ww], bq_sb[b])

    q_half("x1q", 1, 0)
    vt1 = make_vt(x1_sb, 1)
    q_half("x2q", 2, 32)
    vt2 = make_vt(x2_sb, 2)

    # ---- cat_pad buffer [128, 4, 35*66], zeroed ----
    cat = big.tile(BIG_SHAPE_BYTES, F32R, tag="big")
    catv = cat[:].rearrange("p (i f) -> p i f", i=4)
    cat_r = cat[:].rearrange("p (i r w) -> p i r w", i=4, w=WPAD)
    nc.gpsimd.memset(cat[:].bitcast(F32), 0.0)

    # ---- attention branches ----
    for b, (kp, vtv, xr_name) in enumerate(
            [(k_sb[1], vt1, "x1r"), (k_sb[2], vt2, "x2r")]):
        for n0, nw in ATT_BLOCKS:
            nsub = nw // P
            g = 1024 // nw  # m-iters per exp group (4 for nw=256, 8 for 128)
            av = psA.tile([P, 1024], F32, tag="av")

            def flush_av(pend, av=av, vtv=vtv, nw=nw, nsub=nsub):
                g0, ex = pend
                for u in range(1024 // nw):
                    pmi = g0 + u
                    for j in range(nsub):
                        _mm(nc, av[:, ds(j * 512, 258)],
                            ex[:, ds(u * nw + j * P, P)], vtv[:, pmi, :],
                            start=(pmi == 0), stop=(pmi == 31))

            pend = None
            for g0 in range(0, 32, g):
                sc = psS.tile([P, 1024], F32, tag="sc")
                for u in range(g):
                    mi = g0 + u
                    _mm(nc, sc[:, ds(u * nw, nw)],
                        kp[0:64, ts(mi, P)], qp[0:64, ds(n0, nw)],
                        start=True, stop=True)
                ex = expp.tile([P, 1024], F32R, tag="ex")
                nc.scalar.activation(ex, sc, mybir.ActivationFunctionType.Exp)
                if pend is not None:
                    flush_av(pend)
                pend = (g0, ex)
            flush_av(pend)

            # epilogue per n-chunk of 128; transposes reuse the consumed AV bank
            for j in range(nsub):
                nch = n0 // P + j
                rs = scalp.tile([P, 1], F32, tag="rs")
                nc.vector.reciprocal(rs, av[:, ds(j * 512 + 256, 1)])
                nc.vector.tensor_mul(out=rs, in0=rs,
                                     in1=maskg_sb[:, ds(nch, 1)])
                nt = normp.tile([P, 256], F32, tag="nt")
                nc.vector.tensor_scalar_mul(nt, av[:, ds(j * 512, 256)], rs)
                rt = resp.tile([P, 2, P], F32, tag="rt")
                nc.sync.dma_start(
                    out=rt,
                    in_=t[xr_name][:].rearrange("(cc p) n -> p cc n", p=P)
                    [:, :, ts(nch, P)])
                for cc in range(2):
                    tp = av[:, ds(j * 512 + cc * P, P)]
                    nc.tensor.transpose(tp, nt[:, ts(cc, P)], idt)
                    nc.vector.tensor_add(
                        out=cat_r[:, 2 * b + cc, ds(2 * nch, 2), ds(1, 64)],
                        in0=tp.rearrange("p (r w) -> p r w", w=64),
                        in1=rt[:, cc, :].rearrange("p (r w) -> p r w", w=64))

        # write out this branch's attention output (rows 1..33 = the 32 real rows)
        ov = t[f"o{b + 1}"][:].rearrange("(cc p) h w -> p cc h w", p=P)
        for cc in range(2):
            nc.sync.dma_start(out=ov[:, cc],
                              in_=cat_r[:, 2 * b + cc, ds(1, 32), ds(1, 64)])

    # ---- conv 3x3 + BN + ReLU ----
    feat = big.tile(BIG_SHAPE_BYTES, F32, tag="big")
    featv = feat[:, : 2 * 2112].rearrange("p (o f) -> p o f", o=2)
    feat_r = feat[:, : 2 * 2112].rearrange("p (o r w) -> p o r w", o=2, w=WPAD)
    for oc in range(2):
        avc1 = psA.tile([P, 1024], F32, tag="av")
        avc2 = psA.tile([P, 1024], F32, tag="av")
        last = psS.tile([P, 1024], F32, tag="sc")

        def conv_dst(wi, ww, avc1=avc1, avc2=avc2, last=last):
            if wi < 2:
                return avc1[:, ds(wi * 512, ww)]
            if wi < 4:
                return avc2[:, ds((wi - 2) * 512, ww)]
            return last[:, :ww]

        wts = {}
        for ic in range(4):
            for tap in range(9):
                wt = wcp.tile([P, P], F32R, tag="wt", name=f"wt{oc}_{ic}_{tap}")
                nc.sync.dma_start(
                    out=wt, in_=t["wct"][tap // 3, tap % 3,
                                         ts(ic, P), ts(oc, P)])
                wts[(ic, tap)] = wt
        for ic in range(4):
            for tap in range(9):
                off = (tap // 3) * WPAD + (tap % 3) - 1
                for wi, (ws, ww) in enumerate(CONV_WINS):
                    _mm(nc, conv_dst(wi, ww), wts[(ic, tap)],
                        catv[:, ic, ds(ws + off, ww)],
                        start=(ic == 0 and tap == 0),
                        stop=(ic == 3 and tap == 8))
        for wi, (ws, ww) in enumerate(CONV_WINS):
            nc.scalar.activation(featv[:, oc, ds(ws, ww)], conv_dst(wi, ww),
                                 mybir.ActivationFunctionType.Relu,
                                 bias=cbeta_sb[:, ds(oc, 1)],
                                 scale=cinv_sb[:, ds(oc, 1)])
    fv = t["feat"][:].rearrange("(cc p) h w -> p cc h w", p=P)
    for oc in range(2):
        nc.sync.dma_start(out=fv[:, oc], in_=feat_r[:, oc, :, ds(1, 64)])


def _build():
    if "nc" in _CACHE:
        return _CACHE["nc"]
    nc = bacc.Bacc(None, target_bir_lowering=False)
    t = _declare_io(nc)
    from contextlib import ExitStack
    with tile.TileContext(nc) as tc, ExitStack() as ctx:
        _emit(nc, tc, t, ctx)
    nc.finalize()
    _CACHE["nc"] = nc
    return nc


def _prep_host(inputs):
    d = {k: np.ascontiguousarray(np.asarray(v, np.float32)) for k, v in inputs.items()}
    gamma = float(d["gamma"].reshape(-1)[0])
    inv = d["bn_scale"] / np.sqrt(d["bn_var"] + EPS)
    beta = d["bn_bias"] - d["bn_mean"] * inv

    def chunked(w):  # [256, o] -> [128, 2, o]
        return np.ascontiguousarray(w.reshape(2, P, -1).transpose(1, 0, 2))

    shared = {
        "wq1T": chunked(d["wq1"].T), "wq2T": chunked(d["wq2"].T),
        "wk1T": chunked(d["wk1"].T), "wk2T": chunked(d["wk2"].T),
        "wv1T": chunked(d["wv1"].T), "wv2T": chunked(d["wv2"].T),
        "bq1": d["bq1"].reshape(32, 1).copy(), "bq2": d["bq2"].reshape(32, 1).copy(),
        "bk1": d["bk1"].reshape(64, 1).copy(), "bk2": d["bk2"].reshape(64, 1).copy(),
        "cinv": np.ascontiguousarray(inv.reshape(2, P).T),
        "cbeta": np.ascontiguousarray(beta.reshape(2, P).T),
        "wct": np.ascontiguousarray(d["w_cat"].transpose(2, 3, 1, 0)),
        "ident": np.eye(P, dtype=np.float32),
    }
    gbv = {1: gamma * d["bv1"], 2: gamma * d["bv2"]}

    in_maps = []
    for core in range(8):
        s, half = core // 2, core % 2
        h0 = 32 * half
        x1 = np.ascontiguousarray(d["input1"][s].reshape(C, M))
        x2 = np.ascontiguousarray(d["input2"][s].reshape(C, M))
        n_lo, n_hi = (h0 - 1) * 64, (h0 + 33) * 64
        lo_pad, hi_pad = max(0, -n_lo), max(0, n_hi - M)
        sl = slice(n_lo + lo_pad, n_hi - hi_pad)

        def pad_slice(x, add=None):
            o = np.zeros((C, NQ), np.float32)
            body = x[:, sl]
            if add is not None:
                body = body + add[:, None]
            o[:, lo_pad:NQ - hi_pad] = body
            return o

        maskg = np.zeros(NQ, np.float32)
        maskg[lo_pad:NQ - hi_pad] = gamma
        m = dict(shared)
        m.update({
            "x1": x1, "x2": x2,
            "x1q": pad_slice(x1), "x2q": pad_slice(x2),
            "x1r": pad_slice(x1, gbv[1]), "x2r": pad_slice(x2, gbv[2]),
            "maskg": np.ascontiguousarray(maskg.reshape(17, P).T),
        })
        in_maps.append(m)
    return in_maps


def _run_cached_pjrt(nc, in_maps):
    """run_bass_via_pjrt equivalent with the traced/jitted executable cached
    across kernel() calls (run_bass_via_pjrt rebuilds it every call)."""
    import jax
    import numpy as _np
    from jax.sharding import Mesh, PartitionSpec
    from jax.experimental.shard_map import shard_map
    from concourse import bass2jax, mybir as _mb

    n_cores = len(in_maps)
    if "pjrt" not in _CACHE:
        bass2jax.install_neuronx_cc_hook()
        in_names, out_names, out_avals, zero_shapes = [], [], [], []
        for alloc in nc.m.functions[0].allocations:
            if not isinstance(alloc, _mb.MemoryLocationSet):
                continue
            name = alloc.memorylocations[0].name
            if alloc.kind == "ExternalInput":
                if nc.partition_id_tensor is None or \
                        name != nc.partition_id_tensor.name:
                    in_names.append(name)
            elif alloc.kind == "ExternalOutput":
                out_names.append(name)
                shape = tuple(alloc.tensor_shape)
                dtype = _mb.dt.np(alloc.dtype)
                out_avals.append(jax.core.ShapedArray(shape, dtype))
                zero_shapes.append((shape, dtype))
        n_params = len(in_names)
        all_names = in_names + out_names
        pid_name = nc.partition_id_tensor.name if nc.partition_id_tensor else None
        if pid_name is not None:
            all_names = all_names + [pid_name]

        def _body(*args):
            operands = list(args)
            if pid_name is not None:
                operands.append(bass2jax.partition_id_tensor())
            outs = bass2jax._bass_exec_p.bind(
                *operands,
                out_avals=tuple(out_avals),
                in_names=tuple(all_names),
                out_names=tuple(out_names),
                lowering_input_output_aliases=(),
                sim_require_finite=True,
                sim_require_nnan=True,
                nc=nc,
            )
            return tuple(outs)

        devices = jax.devices()[:n_cores]
        mesh = Mesh(_np.asarray(devices), ("core",))
        n_outs = len(out_names)
        sharded = jax.jit(
            shard_map(_body, mesh=mesh,
                      in_specs=(PartitionSpec("core"),) * (n_params + n_outs),
                      out_specs=(PartitionSpec("core"),) * n_outs,
                      check_rep=False),
            donate_argnums=tuple(range(n_params, n_params + n_outs)),
            keep_unused=True,
        )
        _CACHE["pjrt"] = (sharded, in_names, out_names, out_avals, zero_shapes)

    sharded, in_names, out_names, out_avals, zero_shapes = _CACHE["pjrt"]
    n_cores_ax = len(in_maps)
    concat_in = [
        _np.concatenate([_np.asarray(in_maps[c][nm]) for c in range(n_cores_ax)], axis=0)
        for nm in in_names
    ]
    concat_zeros = [
        _np.zeros((n_cores_ax * s[0], *s[1:]), d) for s, d in zero_shapes
    ]
    out_arrs = sharded(*concat_in, *concat_zeros)
    return [
        {nm: _np.asarray(out_arrs[i]).reshape(n_cores_ax, *out_avals[i].shape)[c]
         for i, nm in enumerate(out_names)}
        for c in range(n_cores_ax)
    ]


def kernel(**inputs):
    nc = _build()
    in_maps = _prep_host(inputs)
    try:
        results = _run_cached_pjrt(nc, in_maps)
    except Exception:
        _CACHE.pop("pjrt", None)
        res = run_bass_kernel_spmd(nc, in_maps, core_ids=list(range(8)))
        _CACHE["last_results"] = res
        results = res.results
    feat = np.zeros((4, C, 64, 64), np.float32)
    o1 = np.zeros((4, C, 64, 64), np.float32)
    o2 = np.zeros((4, C, 64, 64), np.float32)
    for core in range(8):
        s, half = core // 2, core % 2
        r = results[core]
        feat[s, :, 32 * half:32 * half + 32] = r["feat"]
        o1[s, :, 32 * half:32 * half + 32] = r["o1"]
        o2[s, :, 32 * half:32 * half + 32] = r["o2"]
    return (feat, o1, o2)



# revision 21
# speedup vs baseline: 1.9276x; 1.9276x over previous
"""Trainium2 Bass kernel for nn_CrossAtt (dual cross-attention + 3x3 conv + BN + ReLU).

Sharding: 8 cores = (sample s in 0..3) x (h-half in 0..1). Each core computes
its 32 output rows plus a 1-row attention halo on each side (34 rows = 2176
query positions, host-zero-padded so the program is SPMD-uniform), then runs
the 3x3 conv locally. No collectives.

v2: fp8e4 + DoubleRow perf mode on the attention path (projections, scores,
AV) — 2 stacked k-tiles per pass at 0.5 cycles/row = 4x fp32r throughput.
Precision is safe because gamma=0.1 attenuates the attention output against
the exact fp32 residual. The 3x3 conv keeps an exact f32r moving operand
(cat) with bf16 stationary weights (BN inv folded in host-side), and its
matmuls are interleaved into the ACT-bound attention phase chunk-by-chunk so
the PE never sits behind the softmax exp stream. Softmax denominator rides
as a 257th ones-column of vT inside the same AV accumulation group. Epilogue
scale/copy work runs on Pool; reciprocal/residual-add on DVE; BN bias + ReLU
fused in one DVE tensor_scalar (add, max).
"""
import sys

if "/opt/trn_rl_repo" not in sys.path:
    sys.path.insert(0, "/opt/trn_rl_repo")

import numpy as np

import concourse.bass as bass
import concourse.bacc as bacc
import concourse.mybir as mybir
import concourse.tile as tile
from concourse.bass import ds, ts
from concourse.bass_utils import run_bass_kernel_spmd

F32 = mybir.dt.float32
F32R = mybir.dt.float32r
BF16 = mybir.dt.bfloat16
F8 = mybir.dt.float8e4
DR = mybir.MatmulPerfMode.DoubleRow
EXP = mybir.ActivationFunctionType.Exp
ADD = mybir.AluOpType.add
MAX = mybir.AluOpType.max
EPS = 1e-5
P = 128
C = 256          # channels
M = 4096         # key/value positions (64*64)
NQ = 2176        # query positions per core (34 rows * 64), host padded
NROWS = 35       # cat_pad rows (34 data + 1 zero)
WPAD = 66        # cat_pad row width (64 + 2 zero cols)
NCHK = 17        # 128-query chunks
BLOCKS = [(i * 128, 128) for i in range(NCHK)]

_CACHE = {}


def _mm(nc, out, lhsT, rhs, **kw):
    nc.tensor.matmul(out, lhsT, rhs, **kw)


def _declare_io(nc):
    t = {}
    inp = lambda name, shape, dt=F32: t.__setitem__(
        name, nc.dram_tensor(name, shape, dt, kind="ExternalInput"))
    out = lambda name, shape, dt=F32: t.__setitem__(
        name, nc.dram_tensor(name, shape, dt, kind="ExternalOutput"))
    inp("x8_1", [C, M], F8); inp("x8_2", [C, M], F8)
    inp("xq8_1", [C, NQ], F8); inp("xq8_2", [C, NQ], F8)
    inp("x1r", [C, NQ]); inp("x2r", [C, NQ])
    inp("maskg", [P, NCHK])
    inp("wq8_1", [P, 2, 32], F8); inp("wq8_2", [P, 2, 32], F8)
    inp("wk8_1", [P, 2, 64], F8); inp("wk8_2", [P, 2, 64], F8)
    inp("wv8_1", [P, 2, C], F8); inp("wv8_2", [P, 2, C], F8)
    inp("bqf", [32, 2, 512])
    inp("bkf1", [32, 2, 512]); inp("bkf2", [32, 2, 512])
    inp("cbeta", [P, 2])
    # conv weights pre-scaled by BN inv, tile layout [p, tap, ic, oc, o]
    inp("wc16", [P, 9, 4, 2, P], BF16)
    inp("ident16", [P, P], BF16)
    out("feat", [C, 32, 64]); out("o1", [C, 32, 64], BF16); out("o2", [C, 32, 64], BF16)
    return t


def _emit(nc, tc, t, ctx):
    sing = ctx.enter_context(tc.tile_pool(name="sing", bufs=1))
    xp = ctx.enter_context(tc.tile_pool(name="xp", bufs=1))
    kq = ctx.enter_context(tc.tile_pool(name="kq", bufs=1))
    vtp = ctx.enter_context(tc.tile_pool(name="vtp", bufs=1))
    expp = ctx.enter_context(tc.tile_pool(name="expp", bufs=3))
    ntp = ctx.enter_context(tc.tile_pool(name="ntp", bufs=2))
    scalp = ctx.enter_context(tc.tile_pool(name="scalp", bufs=4))
    catp = ctx.enter_context(tc.tile_pool(name="catp", bufs=1))
    fcp = ctx.enter_context(tc.tile_pool(name="fcp", bufs=2))
    psS = ctx.enter_context(tc.tile_pool(name="psS", bufs=2, space="PSUM"))
    psA = ctx.enter_context(tc.tile_pool(name="psA", bufs=2, space="PSUM"))
    psC = ctx.enter_context(tc.tile_pool(name="psC", bufs=2, space="PSUM"))

    # ---- constants / weights ----
    idt = sing.tile([P, P], BF16, tag="idt")
    nc.sync.dma_start(out=idt, in_=t["ident16"][:])
    wq8, wk8, wv8 = {}, {}, {}
    for b in (1, 2):
        wq8[b] = sing.tile([P, 2, 32], F8, tag=f"wq{b}", name=f"wq{b}")
        nc.sync.dma_start(out=wq8[b], in_=t[f"wq8_{b}"][:])
        wk8[b] = sing.tile([P, 2, 64], F8, tag=f"wk{b}", name=f"wk{b}")
        nc.sync.dma_start(out=wk8[b], in_=t[f"wk8_{b}"][:])
        wv8[b] = sing.tile([P, 2, C], F8, tag=f"wv{b}", name=f"wv{b}")
        nc.sync.dma_start(out=wv8[b], in_=t[f"wv8_{b}"][:])
    bqf_sb = sing.tile([32, 2, 512], F32, tag="bqf")
    nc.sync.dma_start(out=bqf_sb, in_=t["bqf"][:])
    bkf_sb = {}
    for b in (1, 2):
        bkf_sb[b] = sing.tile([32, 2, 512], F32, tag=f"bkf{b}", name=f"bkf{b}")
        nc.sync.dma_start(out=bkf_sb[b], in_=t[f"bkf{b}"][:])
    maskg_sb = sing.tile([P, NCHK], F32, tag="maskg")
    nc.sync.dma_start(out=maskg_sb, in_=t["maskg"][:])
    cbeta_sb = sing.tile([P, 2], F32, tag="cbeta")
    nc.sync.dma_start(out=cbeta_sb, in_=t["cbeta"][:])
    neg2 = sing.tile([P, 1], F32, tag="neg2")
    nc.vector.memset(neg2, -2.0)

    # ---- inputs, in priority order for the pipeline start ----
    x8, xq8, xr = {}, {}, {}
    for b in (1, 2):
        x8[b] = xp.tile([P, 2, M], F8, tag=f"x8{b}", name=f"x8{b}")
        xq8[b] = xp.tile([P, 2, NQ], F8, tag=f"xq8{b}", name=f"xq8{b}")
        xr[b] = xp.tile([P, 2, NQ], F32, tag=f"xr{b}", name=f"xr{b}")
    x8src = {b: t[f"x8_{b}"][:].rearrange("(kc p) n -> p kc n", p=P)
             for b in (1, 2)}
    for c0 in range(0, M, 2048):
        nc.sync.dma_start(out=x8[1][:, :, ds(c0, 2048)],
                          in_=x8src[1][:, :, ds(c0, 2048)])
    for b in (1, 2):
        srcq = t[f"xq8_{b}"][:].rearrange("(kc p) n -> p kc n", p=P)
        nc.sync.dma_start(out=xq8[b][:, :, 0:1088], in_=srcq[:, :, 0:1088])
        nc.sync.dma_start(out=xq8[b][:, :, 1088:NQ], in_=srcq[:, :, 1088:NQ])
    for c0 in range(0, M, 2048):
        nc.sync.dma_start(out=x8[2][:, :, ds(c0, 2048)],
                          in_=x8src[2][:, :, ds(c0, 2048)])
    for b in (1, 2):
        srcr = t[f"x{b}r"][:].rearrange("(kc p) n -> p kc n", p=P)
        for c0 in range(0, NQ, 1088):
            nc.sync.dma_start(out=xr[b][:, :, ds(c0, 1088)],
                              in_=srcr[:, :, ds(c0, 1088)])
    wcsb = sing.tile([P, 9, 4, 2, P], BF16, tag="wc")
    nc.sync.dma_start(out=wcsb, in_=t["wc16"][:])

    # ---- cat buffer [128, 4, 35*66] f32r, zeroed ----
    cat = catp.tile([P, 4, NROWS * WPAD], BF16, tag="cat")
    nc.gpsimd.memset(cat[:], 0.0)
    cat_r = cat[:].rearrange("p i (r w) -> p i r w", w=WPAD)


    # ---- projections (fp8 DoubleRow), batched conversions ----
    kf = {b: kq.tile([32, 2, M], F8, tag=f"kf{b}", name=f"kf{b}") for b in (1, 2)}
    qf = kq.tile([32, 2, NQ], F8, tag="qf")
    vt = {b: vtp.tile([P, 16, 2, 258], F8, tag=f"vt{b}", name=f"vt{b}")
          for b in (1, 2)}

    def emit_kproj_wide(b, w0):
        # two 256-col windows per psum tile on the scores ring (prologue only)
        ps = psS.tile([P, 1024], F32, tag="sc", name=f"kw{b}_{w0}")
        for wi in range(2):
            for u in range(2):
                _mm(nc, ps[0:32, ds(u * 512 + wi * 256, 256)],
                    wk8[b][:, :, ds(32 * u, 32)],
                    x8[b][:, :, ds(w0 + wi * 256, 256)],
                    start=True, stop=True, perf_mode=DR)
        nc.vector.tensor_add(
            out=kf[b][:, :, ds(w0, 512)],
            in0=ps[0:32, :].rearrange("p (u f) -> p u f", u=2),
            in1=bkf_sb[b])

    def emit_qproj_wide():
        # query windows 0-3 in one psum tile
        ps = psS.tile([P, 1024], F32, tag="sc", name="qw0")
        for b in (1, 2):
            for wi in range(4):
                _mm(nc, ps[0:32, ds((b - 1) * 512 + wi * 128, 128)],
                    wq8[b][:, :, :], xq8[b][:, :, ds(wi * 128, 128)],
                    start=True, stop=True, perf_mode=DR)
        nc.vector.tensor_add(
            out=qf[:, :, 0:512],
            in0=ps[0:32, :].rearrange("p (u f) -> p u f", u=2),
            in1=bqf_sb)

    def emit_qproj_tile(w):
        # windows w, w+1 (one for the tail window)
        nwin = 2 if w + 1 < NCHK else 1
        ps = psC.tile([P, 512], F32, tag="cps", name=f"qp{w}")
        for b in (1, 2):
            for wi in range(nwin):
                _mm(nc, ps[0:32, ds((b - 1) * 256 + wi * 128, 128)],
                    wq8[b][:, :, :], xq8[b][:, :, ds((w + wi) * 128, 128)],
                    start=True, stop=True, perf_mode=DR)
        tot = nwin * 128
        nc.vector.tensor_add(
            out=qf[:, :, ds(w * 128, tot)],
            in0=ps[0:32, :].rearrange("p (u f) -> p u f", u=2)[:, :, 0:tot],
            in1=bqf_sb[:, :, 0:tot])

    def emit_vproj_wide(b, mi2, eng):
        # pairs mi2, mi2+1 in one psum tile; cast on DVE or ACT (Pool
        # cannot read PSUM)
        ps = psS.tile([P, 1024], F32, tag="sc", name=f"vw{b}_{mi2}")
        for pr in range(2):
            for u in range(2):
                _mm(nc, ps[:, ds(pr * 512 + u * 256, 256)],
                    x8[b][:, :, ts(2 * (mi2 + pr) + u, P)],
                    wv8[b][:, :, :], start=True, stop=True, perf_mode=DR)
        dst = vt[b][:, ds(mi2, 2), :, 0:256]
        srcv = ps[:, :].rearrange("p (pr u f) -> p pr u f", pr=2, u=2)
        if eng == "act":
            nc.scalar.copy(out=dst, in_=srcv)
        else:
            nc.vector.tensor_copy(out=dst, in_=srcv)

    # prologue: k1 + q windows 0-3 + v1, wide tiles on the scores ring
    # (free until the stream starts). v1 casts go to ACT, which is idle
    # before the first exp. k2/v2 interleave into the stream; the q tail
    # windows drip through the conv-pool ring.
    for b in (1, 2):
        nc.vector.memset(vt[b][:, :, :, 256:258], 1.0)
    for w0 in range(0, M, 512):
        emit_kproj_wide(1, w0)
    emit_qproj_wide()
    for mi2 in range(0, 16, 2):
        emit_vproj_wide(1, mi2, "act")
    wide_queue = []
    for w0 in range(0, M, 512):
        wide_queue.append(lambda w0=w0: emit_kproj_wide(2, w0))
    for j, mi2 in enumerate(range(0, 16, 2)):
        wide_queue.append(lambda mi2=mi2, e=("act" if j % 2 else "dve"):
                          emit_vproj_wide(2, mi2, e))
    proj_queue = []
    for w in range(4, NCHK, 2):
        proj_queue.append(lambda w=w: emit_qproj_tile(w))

    # ---- conv chunk machinery (spread through the attention stream) ----
    fv = t["feat"][:].rearrange("(cc p) h w -> p cc h w", p=P)
    conv_queue = []

    def unlock_conv_chunk(c):
        pc = {}

        def mk_mm(oc, ic, tap):
            def emit():
                if oc not in pc:
                    pc[oc] = psC.tile([P, 512], F32, tag="cps",
                                      name=f"cps{c}_{oc}")
                off = (tap // 3) * WPAD + (tap % 3) - 1
                _mm(nc, pc[oc][:, 0:264], wcsb[:, tap, ic, oc, :],
                    cat[:, ic, ds(264 * c + 1 + off, 264)],
                    start=(ic == 0 and tap == 0),
                    stop=(ic == 3 and tap == 8))
            return emit

        def mk_tail():
            def emit():
                fc = fcp.tile([P, 2, 264], F32, tag="fc")
                for oc in range(2):
                    nc.vector.tensor_scalar(fc[:, oc, :], pc[oc][:, 0:264],
                                            cbeta_sb[:, ds(oc, 1)], 0.0,
                                            ADD, MAX)
                fcr = fc[:].rearrange("p o (r w) -> p o r w", w=WPAD)
                for oc in range(2):
                    nc.sync.dma_start(out=fv[:, oc, ds(4 * c, 4), :],
                                      in_=fcr[:, oc, :, ds(0, 64)])
            return emit

        for oc in range(2):
            for ic in range(4):
                for tap in range(9):
                    conv_queue.append(mk_mm(oc, ic, tap))
        conv_queue.append(mk_tail())

    def pop_q(q, n):
        for _ in range(n):
            if q:
                q.pop(0)()

    # ---- streaming attention: block-pairs per branch so branch 2 of a
    # pair starts 8 tiles after branch 1 (time for k2/v2 to land) ----
    stream = []
    for pp in range(0, NCHK, 2):
        blks = [pp] if pp + 1 >= NCHK else [pp, pp + 1]
        for b in (1, 2):
            for i in blks:
                for sci in range(4):
                    stream.append((i, b, i * 128, sci, sci == 3))

    av_tiles = {}
    ov_ap = {b: t[f"o{b}"][:].rearrange("(cc p) h w -> p cc h w", p=P)
             for b in (1, 2)}

    def flush_av(i, b, sci, ex):
        if (i, b) not in av_tiles:
            av_tiles[(i, b)] = psA.tile([P, 512], F32, tag="av",
                                        name=f"av{i}_{b}")
        av = av_tiles[(i, b)]
        exv = ex[:].rearrange("p (pr t c) -> p pr t c", pr=4, t=2)
        for pr in range(4):
            T = sci * 4 + pr
            _mm(nc, av[:, 0:257], exv[:, pr, :, :], vt[b][:, T, :, 0:257],
                start=(T == 0), stop=(T == 15), perf_mode=DR)

    def epilogue(i, b):
        av = av_tiles.pop((i, b))
        avb = av[:].bitcast(BF16)
        nch = i
        rs = scalp.tile([P, 1], F32, tag="rs")
        nc.vector.reciprocal(rs, av[:, ds(256, 1)])
        nc.vector.tensor_mul(out=rs, in0=rs, in1=maskg_sb[:, ds(nch, 1)])
        nt = ntp.tile([P, 256], BF16, tag="nt")
        nc.vector.tensor_scalar_mul(nt, av[:, 0:256], rs)
        for cc in range(2):
            tp = avb[:, ds(P * cc, P)]
            nc.tensor.transpose(tp, nt[:, ts(cc, P)], idt)
            nc.vector.tensor_add(
                out=cat_r[:, 2 * (b - 1) + cc, ds(2 * nch, 2), ds(1, 64)],
                in0=tp.rearrange("p (r w) -> p r w", w=64),
                in1=xr[b][:, cc, ts(nch, P)].rearrange("p (r w) -> p r w", w=64))
        lo = max(2 * nch - 1, 0)
        cnt = min(2 * nch, 31) - lo + 1
        for cc in range(2):
            nc.sync.dma_start(
                out=ov_ap[b][:, cc, ds(lo, cnt), :],
                in_=cat_r[:, 2 * (b - 1) + cc, ds(lo + 1, cnt), ds(1, 64)])
        if b == 2 and i >= 2 and i % 2 == 0:
            unlock_conv_chunk(i // 2 - 1)

    pend = None
    epi_due = []
    for tile_d in stream:
        i, b, n0, sci, last = tile_d
        sc = psS.tile([P, 1024], F32, tag="sc")
        for u in range(8):
            mi = sci * 8 + u
            _mm(nc, sc[:, ds(u * 128, 128)], kf[b][:, :, ts(mi, P)],
                qf[:, :, ds(n0, 128)], start=True, stop=True, perf_mode=DR)
        ex = expp.tile([P, 1024], F8, tag="ex")
        # uniform -2 shift keeps exp within fp8e4 range (softmax-invariant)
        nc.scalar.activation(ex, sc, EXP, bias=neg2)
        if pend is not None:
            pi, pb, pn0, psci, plast = pend[0]
            if psci == 0 and len(epi_due) >= 2:
                # the coming flush opens av group g+2, which reuses the
                # buffer of group g: emit group g's epilogue first
                ei, eb = epi_due.pop(0)
                epilogue(ei, eb)
            flush_av(pi, pb, psci, pend[1])
            if plast:
                epi_due.append((pi, pb))
            pop_q(wide_queue, 2)
            pop_q(proj_queue, 1)
            pop_q(conv_queue, 6)
        pend = (tile_d, ex)
    pi, pb, pn0, psci, plast = pend[0]
    while epi_due:
        ei, eb = epi_due.pop(0)
        epilogue(ei, eb)
    flush_av(pi, pb, psci, pend[1])
    epilogue(pi, pb)
    pop_q(wide_queue, len(wide_queue))
    pop_q(proj_queue, len(proj_queue))
    pop_q(conv_queue, len(conv_queue))


def _build():
    if "nc" in _CACHE:
        return _CACHE["nc"]
    nc = bacc.Bacc(None, target_bir_lowering=False)
    t = _declare_io(nc)
    from contextlib import ExitStack
    with tile.TileContext(nc) as tc, ExitStack() as ctx:
        _emit(nc, tc, t, ctx)
    nc.finalize()
    _CACHE["nc"] = nc
    return nc


def _prep_host(inputs):
    d = {k: np.ascontiguousarray(np.asarray(v, np.float32)) for k, v in inputs.items()}
    f8 = mybir.dt.np(F8)
    bf = mybir.dt.np(BF16)
    gamma = float(d["gamma"].reshape(-1)[0])
    inv = d["bn_scale"] / np.sqrt(d["bn_var"] + EPS)
    beta = d["bn_bias"] - d["bn_mean"] * inv

    def chunked(w):  # [256, o] -> [128, 2, o]
        return np.ascontiguousarray(w.reshape(2, P, -1).transpose(1, 0, 2))

    # conv weights pre-scaled by inv, laid out [p, tap, ic, oc, o]
    wct = (d["w_cat"] * inv[:, None, None, None]).transpose(2, 3, 1, 0)
    # wct[cin, ky, kx, O] -> wc16[p, tap, ic, oc, o]
    wc16 = np.zeros((P, 9, 4, 2, P), np.float32)
    for tap in range(9):
        for ic in range(4):
            for oc in range(2):
                wc16[:, tap, ic, oc, :] = wct[tap // 3, tap % 3,
                                              ic * P:(ic + 1) * P,
                                              oc * P:(oc + 1) * P]
    bqf = np.zeros((32, 2, 512), np.float32)
    bqf[:, 0, :] = d["bq1"][:, None]
    bqf[:, 1, :] = d["bq2"][:, None]
    bkf = {}
    for bi, key in ((1, "bk1"), (2, "bk2")):
        z = np.zeros((32, 2, 512), np.float32)
        z[:, 0, :] = d[key][0:32, None]
        z[:, 1, :] = d[key][32:64, None]
        bkf[bi] = z
    shared = {
        "wq8_1": chunked(d["wq1"].T).astype(f8),
        "wq8_2": chunked(d["wq2"].T).astype(f8),
        "wk8_1": chunked(d["wk1"].T).astype(f8),
        "wk8_2": chunked(d["wk2"].T).astype(f8),
        "wv8_1": chunked(d["wv1"].T).astype(f8),
        "wv8_2": chunked(d["wv2"].T).astype(f8),
        "bqf": bqf, "bkf1": bkf[1], "bkf2": bkf[2],
        "cbeta": np.ascontiguousarray(beta.reshape(2, P).T),
        "wc16": np.ascontiguousarray(wc16).astype(bf),
        "ident16": np.eye(P, dtype=np.float32).astype(bf),
    }
    gbv = {1: gamma * d["bv1"], 2: gamma * d["bv2"]}

    in_maps = []
    for core in range(8):
        s, half = core // 2, core % 2
        h0 = 32 * half
        x1 = np.ascontiguousarray(d["input1"][s].reshape(C, M))
        x2 = np.ascontiguousarray(d["input2"][s].reshape(C, M))
        n_lo, n_hi = (h0 - 1) * 64, (h0 + 33) * 64
        lo_pad, hi_pad = max(0, -n_lo), max(0, n_hi - M)
        sl = slice(n_lo + lo_pad, n_hi - hi_pad)

        def pad_slice(x, add=None):
            o = np.zeros((C, NQ), np.float32)
            body = x[:, sl]
            if add is not None:
                body = body + add[:, None]
            o[:, lo_pad:NQ - hi_pad] = body
            return o

        maskg = np.zeros(NQ, np.float32)
        maskg[lo_pad:NQ - hi_pad] = gamma
        m = dict(shared)
        m.update({
            "x8_1": x1.astype(f8), "x8_2": x2.astype(f8),
            "xq8_1": pad_slice(x1).astype(f8),
            "xq8_2": pad_slice(x2).astype(f8),
            "x1r": pad_slice(x1, gbv[1]), "x2r": pad_slice(x2, gbv[2]),
            "maskg": np.ascontiguousarray(maskg.reshape(NCHK, P).T),
        })
        in_maps.append(m)
    return in_maps


def _run_cached_pjrt(nc, in_maps):
    """run_bass_via_pjrt equivalent with the traced/jitted executable cached
    across kernel() calls (run_bass_via_pjrt rebuilds it every call)."""
    import jax
    import numpy as _np
    from jax.sharding import Mesh, PartitionSpec
    from jax.experimental.shard_map import shard_map
    from concourse import bass2jax, mybir as _mb

    n_cores = len(in_maps)
    if "pjrt" not in _CACHE:
        bass2jax.install_neuronx_cc_hook()
        in_names, out_names, out_avals, zero_shapes = [], [], [], []
        for alloc in nc.m.functions[0].allocations:
            if not isinstance(alloc, _mb.MemoryLocationSet):
                continue
            name = alloc.memorylocations[0].name
            if alloc.kind == "ExternalInput":
                if nc.partition_id_tensor is None or \
                        name != nc.partition_id_tensor.name:
                    in_names.append(name)
            elif alloc.kind == "ExternalOutput":
                out_names.append(name)
                shape = tuple(alloc.tensor_shape)
                dtype = _mb.dt.np(alloc.dtype)
                out_avals.append(jax.core.ShapedArray(shape, dtype))
                zero_shapes.append((shape, dtype))
        n_params = len(in_names)
        all_names = in_names + out_names
        pid_name = nc.partition_id_tensor.name if nc.partition_id_tensor else None
        if pid_name is not None:
            all_names = all_names + [pid_name]

        def _body(*args):
            operands = list(args)
            if pid_name is not None:
                operands.append(bass2jax.partition_id_tensor())
            outs = bass2jax._bass_exec_p.bind(
                *operands,
                out_avals=tuple(out_avals),
                in_names=tuple(all_names),
                out_names=tuple(out_names),
                lowering_input_output_aliases=(),
                sim_require_finite=True,
                sim_require_nnan=True,
                nc=nc,
            )
            return tuple(outs)

        devices = jax.devices()[:n_cores]
        mesh = Mesh(_np.asarray(devices), ("core",))
        n_outs = len(out_names)
        sharded = jax.jit(
            shard_map(_body, mesh=mesh,
                      in_specs=(PartitionSpec("core"),) * (n_params + n_outs),
                      out_specs=(PartitionSpec("core"),) * n_outs,
                      check_rep=False),
            donate_argnums=tuple(range(n_params, n_params + n_outs)),
            keep_unused=True,
        )
        _CACHE["pjrt"] = (sharded, in_names, out_names, out_avals, zero_shapes)

    sharded, in_names, out_names, out_avals, zero_shapes = _CACHE["pjrt"]
    n_cores_ax = len(in_maps)
    concat_in = [
        _np.concatenate([_np.asarray(in_maps[c][nm]) for c in range(n_cores_ax)], axis=0)
        for nm in in_names
    ]
    concat_zeros = [
        _np.zeros((n_cores_ax * s[0], *s[1:]), d) for s, d in zero_shapes
    ]
    out_arrs = sharded(*concat_in, *concat_zeros)
    return [
        {nm: _np.asarray(out_arrs[i]).reshape(n_cores_ax, *out_avals[i].shape)[c]
         for i, nm in enumerate(out_names)}
        for c in range(n_cores_ax)
    ]


def kernel(**inputs):
    nc = _build()
    in_maps = _prep_host(inputs)
    try:
        results = _run_cached_pjrt(nc, in_maps)
    except Exception:
        _CACHE.pop("pjrt", None)
        res = run_bass_kernel_spmd(nc, in_maps, core_ids=list(range(8)))
        _CACHE["last_results"] = res
        results = res.results
    feat = np.zeros((4, C, 64, 64), np.float32)
    o1 = np.zeros((4, C, 64, 64), np.float32)
    o2 = np.zeros((4, C, 64, 64), np.float32)
    for core in range(8):
        s, half = core // 2, core % 2
        r = results[core]
        feat[s, :, 32 * half:32 * half + 32] = r["feat"]
        o1[s, :, 32 * half:32 * half + 32] = r["o1"]
        o2[s, :, 32 * half:32 * half + 32] = r["o2"]
    return (feat, o1, o2)
